# revision 25
# baseline (speedup 1.0000x reference)
"""Trainium2 Bass kernel for nn_MultiHeadAttention_69106023793143.

Reference computation (B=4, S=2048, D=1024, H=16, HD=64):
    qh = split_heads(q @ Wq + bq); kh, vh likewise
    out = merge_heads(sigmoid((qh @ kh^T) / sqrt(HD)) @ vh)

Sharding (8 cores): core c handles batch b = c//2 and the half = c%2 slice of
the feature axis (512 features = 8 heads).  Projections are tensor-parallel on
the output dim of Wq/Wk/Wv; attention is head-parallel.  The final [B,S,D]
output is assembled host-side from the per-core [512, 2048] transposed blocks.

Device strategy per core:
  - Host pre-transposes q/k/v to x^T [D, S] so the contraction dim (features)
    lands on SBUF partitions with plain contiguous DMAs — no on-device
    transposes anywhere.
  - Q^T, K^T computed as W^T-slice @ x^T -> [of, tok] layout (head dim on
    partitions), V computed natural [tok, of].
  - scores^T[k, q] = Kh^T.T @ Qh^T via row-tiled (K=64) matmul pairs: two
    heads run concurrently on disjoint PE row groups (tile_position (0,0) /
    (64,0)).
  - sigmoid on ScalarE directly from PSUM, two banks (2 x [128,512]) per
    ACTIVATE, with the 1/sqrt(HD) scale folded into ACT's free affine.
  - out^T[d, q] accumulated in PSUM over the 16 k-tiles; one [64,512]
    accumulator bank per head at PSUM partition 0 (col-tiled dst partitions
    != 0 are rejected by this toolchain's ISA checks).
  - All matmuls run in float32r (fp32 storage, FP22 multiply) for full PE
    rate; PSUM accumulation is fp32.
  - PSUM budget (8 banks): 2 score slots x 2 banks + 2 projection
    accumulators (vacc0/1) + 2 output accumulators (acc0/1). The dedicated
    projection accumulators let V / per-chunk Q projections pipeline into PE
    slack underneath the sigmoid stream.
  - Nonzero biases are folded in by augmenting the contraction dim with a
    ones-row (host-side, KT=9); with zero biases (the spec'd case) no
    padding is used.
"""

import sys

if "/opt/trn_rl_repo" not in sys.path:
    sys.path.insert(0, "/opt/trn_rl_repo")

from contextlib import ExitStack

import numpy as np

import concourse.tile as tile
from concourse import bacc, mybir
from concourse.bass_utils import run_bass_kernel_spmd

B, S, D, H = 4, 2048, 1024, 16
HD = D // H  # 64
OF = D // 2  # 512 features (8 heads) per core
N_CORES = 8
P = 128
TOK_T = S // P  # 16 token tiles
QC = S // 512  # 4 query chunks of 512
HP = 4  # head pairs per core
F32 = mybir.dt.float32
F32R = mybir.dt.float32r

# number of (kt, head) S-tile jobs per (head-pair, q-chunk), grouped in
# waves of 3 PSUM banks per ACTIVATE
WAVE = 2

_cache: dict = {}

# results of the most recent run (exec time etc.), for test harnesses
last_results = None


def _build(KT: int):
    """Build the SPMD Bass program. KT = contraction k-tiles (8, or 9 when
    biases are folded in via an augmented ones-row)."""
    nc = bacc.Bacc("TRN2", target_bir_lowering=False, debug=False,
                   num_devices=N_CORES, name="mha_sig")

    KA = KT * P  # augmented contraction size
    xq = nc.dram_tensor("xq", [KA, S], F32R, kind="ExternalInput")
    xk = nc.dram_tensor("xk", [KA, S], F32R, kind="ExternalInput")
    xv = nc.dram_tensor("xv", [KA, S], F32R, kind="ExternalInput")
    wq = nc.dram_tensor("wq", [KA, OF], F32R, kind="ExternalInput")
    wk = nc.dram_tensor("wk", [KA, OF], F32R, kind="ExternalInput")
    wv = nc.dram_tensor("wv", [KA, OF], F32R, kind="ExternalInput")
    o_t = nc.dram_tensor("o_t", [OF, S], F32, kind="ExternalOutput")

    xq_r = xq.rearrange("(kt p) t -> p kt t", p=P)
    xk_r = xk.rearrange("(kt p) t -> p kt t", p=P)
    xv_r = xv.rearrange("(kt p) t -> p kt t", p=P)

    # the augmented (KT=9) layout is bigger; drop x-chunk buffering to fit
    xbufs = 3 if KT == 8 else 2

    with tile.TileContext(nc) as tc:
        with ExitStack() as ctx:
            persist = ctx.enter_context(tc.tile_pool(name="persist", bufs=1))
            wpool = ctx.enter_context(tc.tile_pool(name="wpool", bufs=1))
            xpool = ctx.enter_context(tc.tile_pool(name="xpool", bufs=2))
            ps_pool = ctx.enter_context(
                tc.tile_pool(name="ps_pool", bufs=2, space="PSUM"))
            apool = ctx.enter_context(tc.tile_pool(name="apool", bufs=3))
            opool = ctx.enter_context(tc.tile_pool(name="opool", bufs=1))

            # --- persistent weights + projection outputs ---
            # (each W is DMA'd right before the projection phase that uses it
            # so the serial prefix DMA stream isn't front-loaded with all
            # three weight tensors)
            wk_sb = persist.tile([P, KT, OF], F32R)
            nc.sync.dma_start(wk_sb[:], wk.rearrange("(kt p) n -> p kt n", p=P))
            wv_sb = persist.tile([P, KT, OF], F32R)
            wq_sb = persist.tile([P, KT, OF], F32R)

            # K^T / Q^T: [of-in-tile, of-tile, tok];  V: [tok-in-tile, kt, of]
            kt_sb = persist.tile([P, HP, S], F32R)
            v_sb = persist.tile([P, TOK_T, OF], F32R)

            def proj_transposed(x_r, w_sb, dst, tc_idx, label):
                """dst[:, m, tc*512:+512] = (W-slice).T @ x-chunk  ([of, tok]);
                for label=="q", dst is a per-chunk [P, HP, 512] tile and the
                tok axis is not offset."""
                # q chunks get their own slot so the first q-chunk's DMA is
                # not serialized behind all the k chunks in the pool rotation
                x_tile = xpool.tile([P, KT, 512], F32R,
                                    tag="xq" if label == "q" else "xchunk",
                                    bufs=1 if label == "q" else xbufs,
                                    name=f"x_{label}_{tc_idx}")
                nc.sync.dma_start(x_tile[:],
                                  x_r[:, :, tc_idx * 512:(tc_idx + 1) * 512])
                for m in range(HP):
                    ps = ps_pool.tile([P, 512], F32, tag=f"vacc{m % 2}", bufs=1,
                                      name=f"ps_{label}_{tc_idx}_{m}")
                    for kt in range(KT):
                        nc.tensor.matmul(
                            ps[:],
                            lhsT=w_sb[:, kt, m * P:(m + 1) * P],
                            rhs=x_tile[:, kt, :],
                            start=(kt == 0),
                            stop=(kt == KT - 1),
                        )
                    if label == "q":
                        nc.vector.tensor_copy(out=dst[:, m, :], in_=ps[:])
                    else:
                        nc.vector.tensor_copy(
                            out=dst[:, m, tc_idx * 512:(tc_idx + 1) * 512],
                            in_=ps[:])

            def proj_v(tc_idx):
                """v_sb[:, tc*4+m, :] = x-tile.T @ Wv  ([tok, of])"""
                x_tile = xpool.tile([P, KT, 512], F32R, tag="xchunk", bufs=xbufs,
                                    name=f"x_v_{tc_idx}")
                nc.sync.dma_start(x_tile[:],
                                  xv_r[:, :, tc_idx * 512:(tc_idx + 1) * 512])
                for m in range(4):
                    ps = ps_pool.tile([P, 512], F32, tag=f"vacc{m % 2}", bufs=1,
                                      name=f"ps_v_{tc_idx}_{m}")
                    for kt in range(KT):
                        nc.tensor.matmul(
                            ps[:],
                            lhsT=x_tile[:, kt, m * P:(m + 1) * P],
                            rhs=wv_sb[:, kt, :],
                            start=(kt == 0),
                            stop=(kt == KT - 1),
                        )
                    nc.vector.tensor_copy(out=v_sb[:, tc_idx * 4 + m, :], in_=ps[:])

            # K projections first, then Q for the first q-chunk (these gate
            # the first sigmoid waves). V projections are emitted inside the
            # first attention q-chunk: they have their own PSUM tags (vacc*)
            # so they pipeline into PE slack while sigmoids run, and the
            # lagging AV matmuls pick up each V tile as it lands.
            for tc_idx in range(QC):
                proj_transposed(xk_r, wk_sb, kt_sb, tc_idx, "k")
            nc.sync.dma_start(wq_sb[:], wq.rearrange("(kt p) n -> p kt n", p=P))
            qt_tiles = {}
            qt_tiles[0] = xpool.tile([P, HP, 512], F32R, tag="qt", bufs=2,
                                     name="qt_0")
            proj_transposed(xq_r, wq_sb, qt_tiles[0], 0, "q")
            nc.sync.dma_start(wv_sb[:], wv.rearrange("(kt p) n -> p kt n", p=P))

            # jobs per (hp, qc): (kt, head) pairs, kt-major so adjacent jobs
            # alternate PE row groups
            jobs = [(kt, h) for kt in range(TOK_T) for h in range(2)]
            waves = [jobs[i:i + WAVE] for i in range(0, len(jobs), WAVE)]

            # V projections complete before attention so the AV matmuls
            # never starve the sigmoid pipeline (a_t slots are scarce)
            for tc_idx in range(QC):
                proj_v(tc_idx)

            for qc in range(QC):
                if qc > 0:
                    # just-in-time Q projection for the next q-chunk
                    qt_tiles[qc] = xpool.tile([P, HP, 512], F32R, tag="qt",
                                              bufs=2, name=f"qt_{qc}")
                    proj_transposed(xq_r, wq_sb, qt_tiles[qc], qc, "q")

                for hp in range(HP):
                    # per-head output accumulators, both at PSUM partition 0
                    # (col-tiled dst partitions != 0 are rejected by walrus
                    # ISA checks in this toolchain)
                    o_accs = [
                        ps_pool.tile([HD, 512], F32, tag=f"acc{h}", bufs=1,
                                     name=f"oacc{h}_{qc}_{hp}")
                        for h in range(2)
                    ]
                    def emit_avs(wave, a_t):
                        for j, (kt, h) in enumerate(wave):
                            # out^T[d, q] += V-tile.T @ attn^T-tile
                            nc.tensor.matmul(
                                o_accs[h][:],
                                lhsT=v_sb[:, kt,
                                          hp * P + h * HD:hp * P + (h + 1) * HD],
                                rhs=a_t[:, j, :],
                                start=(kt == 0),
                                stop=(kt == TOK_T - 1),
                            )

                    # AV matmuls are emitted one wave behind the scores so the
                    # in-order PE stream never blocks on the current wave's
                    # sigmoid (S(w+1) runs while ACT processes wave w).
                    pending = None
                    for wi, wave in enumerate(waves):
                        st = ps_pool.tile([P, WAVE, 512], F32, tag="scores",
                                       bufs=2, name=f"st_{qc}_{hp}_{wi}")
                        for j, (kt, h) in enumerate(wave):
                            # scores^T tile: [k-tokens, q-tokens] for head
                            # 2hp+h; contraction over d (64 rows)
                            nc.tensor.matmul(
                                st[:, j, :],
                                lhsT=kt_sb[h * HD:(h + 1) * HD, hp,
                                           kt * P:(kt + 1) * P],
                                rhs=qt_tiles[qc][h * HD:(h + 1) * HD, hp, :],
                                start=True,
                                stop=True,
                                tile_position=(h * HD, 0),
                            )
                        a_t = apool.tile([P, WAVE, 512], F32R, tag="a_t",
                                         name=f"a_{qc}_{hp}_{wi}")
                        nc.scalar.activation(
                            out=a_t[:, :len(wave), :],
                            in_=st[:, :len(wave), :],
                            func=mybir.ActivationFunctionType.Sigmoid,
                            scale=1.0 / np.sqrt(HD).item(),
                        )
                        if pending is not None:
                            emit_avs(*pending)
                        pending = (wave, a_t)
                    emit_avs(*pending)
                    o_sb = opool.tile([P, 512], F32, tag="o_sb",
                                      name=f"osb_{qc}_{hp}")
                    nc.vector.tensor_copy(out=o_sb[0:HD, :], in_=o_accs[0][:])
                    nc.vector.tensor_copy(out=o_sb[HD:P, :], in_=o_accs[1][:])
                    nc.sync.dma_start(
                        o_t[hp * P:(hp + 1) * P, qc * 512:(qc + 1) * 512],
                        o_sb[:])

    nc.compile()
    return nc


def _prep_core_inputs(q, k, v, Wq, bq, Wk, bk, Wv, bv, KT):
    """Host-side shard + transpose. Returns in_maps for 8 cores."""
    KA = KT * P
    aug = KA > D

    def x_t(x_b):  # [S, D] -> [KA, S]
        xt = np.ascontiguousarray(x_b.T)  # [D, S]
        if not aug:
            return xt
        out = np.zeros((KA, S), np.float32)
        out[:D] = xt
        out[D] = 1.0
        return out

    def w_slice(W, b, half):  # -> [KA, OF]
        ws = W[:, half * OF:(half + 1) * OF]
        if not aug:
            return np.ascontiguousarray(ws)
        out = np.zeros((KA, OF), np.float32)
        out[:D] = ws
        out[D] = b[half * OF:(half + 1) * OF]
        return out

    xts = {}
    in_maps = []
    for c in range(N_CORES):
        b, half = divmod(c, 2)
        if b not in xts:
            xts[b] = (x_t(q[b]), x_t(k[b]), x_t(v[b]))
        xq_c, xk_c, xv_c = xts[b]
        in_maps.append({
            "xq": xq_c,
            "xk": xk_c,
            "xv": xv_c,
            "wq": w_slice(Wq, bq, half),
            "wk": w_slice(Wk, bk, half),
            "wv": w_slice(Wv, bv, half),
        })
    return in_maps


def kernel(q, k, v, Wq, bq, Wk, bk, Wv, bv):
    global last_results
    q = np.ascontiguousarray(np.asarray(q, np.float32))
    k = np.ascontiguousarray(np.asarray(k, np.float32))
    v = np.ascontiguousarray(np.asarray(v, np.float32))
    Wq = np.asarray(Wq, np.float32)
    Wk = np.asarray(Wk, np.float32)
    Wv = np.asarray(Wv, np.float32)
    bq = np.asarray(bq, np.float32)
    bk = np.asarray(bk, np.float32)
    bv = np.asarray(bv, np.float32)

    aug = any(np.any(b_) for b_ in (bq, bk, bv))
    KT = (D // P) + (1 if aug else 0)

    if KT not in _cache:
        _cache[KT] = _build(KT)
    nc = _cache[KT]

    in_maps = _prep_core_inputs(q, k, v, Wq, bq, Wk, bk, Wv, bv, KT)
    res = run_bass_kernel_spmd(nc, in_maps, core_ids=list(range(N_CORES)))
    last_results = res

    out = np.empty((B, S, D), np.float32)
    for c in range(N_CORES):
        b, half = divmod(c, 2)
        out[b, :, half * OF:(half + 1) * OF] = res.results[c]["o_t"].T
    return out


# revision 28
# speedup vs baseline: 1.0032x; 1.0032x over previous
"""Trainium2 Bass kernel for nn_MultiHeadAttention_69106023793143.

Reference computation (B=4, S=2048, D=1024, H=16, HD=64):
    qh = split_heads(q @ Wq + bq); kh, vh likewise
    out = merge_heads(sigmoid((qh @ kh^T) / sqrt(HD)) @ vh)

Sharding (8 cores): core c handles batch b = c//2 and the half = c%2 slice of
the feature axis (512 features = 8 heads).  Projections are tensor-parallel on
the output dim of Wq/Wk/Wv; attention is head-parallel.  The final [B,S,D]
output is assembled host-side from the per-core [512, 2048] transposed blocks.

Device strategy per core:
  - Host pre-transposes q/k/v to x^T [D, S] so the contraction dim (features)
    lands on SBUF partitions with plain contiguous DMAs — no on-device
    transposes anywhere.
  - Q^T, K^T computed as W^T-slice @ x^T -> [of, tok] layout (head dim on
    partitions), V computed natural [tok, of].
  - scores^T[k, q] = Kh^T.T @ Qh^T via row-tiled (K=64) matmul pairs: two
    heads run concurrently on disjoint PE row groups.
  - sigmoid on ScalarE directly from PSUM (3 banks per ACTIVATE), scale=1/8
    folded into ACT's free affine.
  - out^T[d, q] accumulated in PSUM over the 16 k-tiles via col-tiled (M=64)
    matmul pairs (two heads on disjoint PE col groups).
  - All matmuls run in float32r (fp32 storage, FP22 multiply) for full PE rate.
  - Nonzero biases are folded in by augmenting the contraction dim with a
    ones-row (host-side); with zero biases (the spec'd case) no padding is
    used.
"""

import sys

if "/opt/trn_rl_repo" not in sys.path:
    sys.path.insert(0, "/opt/trn_rl_repo")

from contextlib import ExitStack

import numpy as np

import concourse.tile as tile
from concourse import bacc, mybir
from concourse.bass_utils import run_bass_kernel_spmd

B, S, D, H = 4, 2048, 1024, 16
HD = D // H  # 64
OF = D // 2  # 512 features (8 heads) per core
N_CORES = 8
P = 128
TOK_T = S // P  # 16 token tiles
QC = S // 512  # 4 query chunks of 512
HP = 4  # head pairs per core
F32 = mybir.dt.float32
F32R = mybir.dt.float32r

# number of (kt, head) S-tile jobs per (head-pair, q-chunk), grouped in
# waves of 3 PSUM banks per ACTIVATE
WAVE = 2

_cache: dict = {}

# results of the most recent run (exec time etc.), for test harnesses
last_results = None


def _build(KT: int):
    """Build the SPMD Bass program. KT = contraction k-tiles (8, or 9 when
    biases are folded in via an augmented ones-row)."""
    nc = bacc.Bacc("TRN2", target_bir_lowering=False, debug=False,
                   num_devices=N_CORES, name="mha_sig")

    KA = KT * P  # augmented contraction size
    xq = nc.dram_tensor("xq", [KA, S], F32R, kind="ExternalInput")
    xk = nc.dram_tensor("xk", [KA, S], F32R, kind="ExternalInput")
    xv = nc.dram_tensor("xv", [KA, S], F32R, kind="ExternalInput")
    wq = nc.dram_tensor("wq", [KA, OF], F32R, kind="ExternalInput")
    wk = nc.dram_tensor("wk", [KA, OF], F32R, kind="ExternalInput")
    wv = nc.dram_tensor("wv", [KA, OF], F32R, kind="ExternalInput")
    o_t = nc.dram_tensor("o_t", [OF, S], F32, kind="ExternalOutput")

    xq_r = xq.rearrange("(kt p) t -> p kt t", p=P)
    xk_r = xk.rearrange("(kt p) t -> p kt t", p=P)
    xv_r = xv.rearrange("(kt p) t -> p kt t", p=P)

    with tile.TileContext(nc) as tc:
        with ExitStack() as ctx:
            persist = ctx.enter_context(tc.tile_pool(name="persist", bufs=1))
            wpool = ctx.enter_context(tc.tile_pool(name="wpool", bufs=1))
            xpool = ctx.enter_context(tc.tile_pool(name="xpool", bufs=2))
            ps_pool = ctx.enter_context(
                tc.tile_pool(name="ps_pool", bufs=2, space="PSUM"))
            apool = ctx.enter_context(tc.tile_pool(name="apool", bufs=3))
            opool = ctx.enter_context(tc.tile_pool(name="opool", bufs=1))

            # --- persistent weights + projection outputs ---
            # (each W is DMA'd right before the projection phase that uses it
            # so the serial prefix DMA stream isn't front-loaded with all
            # three weight tensors)
            wk_sb = persist.tile([P, KT, OF], F32R)
            nc.sync.dma_start(wk_sb[:], wk.rearrange("(kt p) n -> p kt n", p=P))
            wv_sb = persist.tile([P, KT, OF], F32R)
            wq_sb = persist.tile([P, KT, OF], F32R)

            # K^T / Q^T: [of-in-tile, of-tile, tok];  V: [tok-in-tile, kt, of]
            kt_sb = persist.tile([P, HP, S], F32R)
            v_sb = persist.tile([P, TOK_T, OF], F32R)

            def proj_transposed(x_r, w_sb, dst, tc_idx, label):
                """dst[:, m, tc*512:+512] = (W-slice).T @ x-chunk  ([of, tok]);
                for label=="q", dst is a per-chunk [P, HP, 512] tile and the
                tok axis is not offset."""
                # q chunks get their own slot so the first q-chunk's DMA is
                # not serialized behind all the k chunks in the pool rotation
                x_tile = xpool.tile([P, KT, 512], F32R,
                                    tag="xq" if label == "q" else "xchunk",
                                    bufs=1 if label == "q" else 3,
                                    name=f"x_{label}_{tc_idx}")
                nc.sync.dma_start(x_tile[:],
                                  x_r[:, :, tc_idx * 512:(tc_idx + 1) * 512])
                for m in range(HP):
                    ps = ps_pool.tile([P, 512], F32, tag=f"vacc{m % 2}", bufs=1,
                                      name=f"ps_{label}_{tc_idx}_{m}")
                    for kt in range(KT):
                        nc.tensor.matmul(
                            ps[:],
                            lhsT=w_sb[:, kt, m * P:(m + 1) * P],
                            rhs=x_tile[:, kt, :],
                            start=(kt == 0),
                            stop=(kt == KT - 1),
                        )
                    if label == "q":
                        nc.vector.tensor_copy(out=dst[:, m, :], in_=ps[:])
                    else:
                        nc.vector.tensor_copy(
                            out=dst[:, m, tc_idx * 512:(tc_idx + 1) * 512],
                            in_=ps[:])

            def proj_v(tc_idx):
                """v_sb[:, tc*4+m, :] = x-tile.T @ Wv  ([tok, of])"""
                x_tile = xpool.tile([P, KT, 512], F32R, tag="xchunk", bufs=3,
                                    name=f"x_v_{tc_idx}")
                nc.sync.dma_start(x_tile[:],
                                  xv_r[:, :, tc_idx * 512:(tc_idx + 1) * 512])
                for m in range(4):
                    ps = ps_pool.tile([P, 512], F32, tag=f"vacc{m % 2}", bufs=1,
                                      name=f"ps_v_{tc_idx}_{m}")
                    for kt in range(KT):
                        nc.tensor.matmul(
                            ps[:],
                            lhsT=x_tile[:, kt, m * P:(m + 1) * P],
                            rhs=wv_sb[:, kt, :],
                            start=(kt == 0),
                            stop=(kt == KT - 1),
                        )
                    nc.vector.tensor_copy(out=v_sb[:, tc_idx * 4 + m, :], in_=ps[:])

            # K projections first, then Q for the first q-chunk (these gate
            # the first sigmoid waves). V projections are emitted inside the
            # first attention q-chunk: they have their own PSUM tags (vacc*)
            # so they pipeline into PE slack while sigmoids run, and the
            # lagging AV matmuls pick up each V tile as it lands.
            for tc_idx in range(QC):
                proj_transposed(xk_r, wk_sb, kt_sb, tc_idx, "k")
            nc.sync.dma_start(wq_sb[:], wq.rearrange("(kt p) n -> p kt n", p=P))
            qt_tiles = {}
            qt_tiles[0] = xpool.tile([P, HP, 512], F32R, tag="qt", bufs=2,
                                     name="qt_0")
            proj_transposed(xq_r, wq_sb, qt_tiles[0], 0, "q")
            nc.sync.dma_start(wv_sb[:], wv.rearrange("(kt p) n -> p kt n", p=P))

            # jobs per (hp, qc): (kt, head) pairs, kt-major so adjacent jobs
            # alternate PE row groups
            jobs = [(kt, h) for kt in range(TOK_T) for h in range(2)]
            waves = [jobs[i:i + WAVE] for i in range(0, len(jobs), WAVE)]

            # Only the first half of V precedes attention: the second half
            # streams in under round 0, staying just ahead of the lagging AV
            # matmuls (V tiles are produced ~1 per 1.8us vs consumed ~1 per
            # 1.1us, and the AVs start a few waves behind the sigmoids).
            proj_v(0)

            for qc in range(QC):
                if qc > 0:
                    # just-in-time Q projection for the next q-chunk
                    qt_tiles[qc] = xpool.tile([P, HP, 512], F32R, tag="qt",
                                              bufs=2, name=f"qt_{qc}")
                    proj_transposed(xq_r, wq_sb, qt_tiles[qc], qc, "q")

                for hp in range(HP):
                    # per-head output accumulators, both at PSUM partition 0
                    # (col-tiled dst partitions != 0 are rejected by walrus
                    # ISA checks in this toolchain)
                    o_accs = [
                        ps_pool.tile([HD, 512], F32, tag=f"acc{h}", bufs=1,
                                     name=f"oacc{h}_{qc}_{hp}")
                        for h in range(2)
                    ]
                    def emit_avs(wave, a_t):
                        for j, (kt, h) in enumerate(wave):
                            # out^T[d, q] += V-tile.T @ attn^T-tile
                            nc.tensor.matmul(
                                o_accs[h][:],
                                lhsT=v_sb[:, kt,
                                          hp * P + h * HD:hp * P + (h + 1) * HD],
                                rhs=a_t[:, j, :],
                                start=(kt == 0),
                                stop=(kt == TOK_T - 1),
                            )

                    # AV matmuls are emitted one wave behind the scores so the
                    # in-order PE stream never blocks on the current wave's
                    # sigmoid (S(w+1) runs while ACT processes wave w).
                    pending = None
                    for wi, wave in enumerate(waves):
                        # V chunks tc1..tc3 are projected just-in-time inside
                        # round 0's wave stream (wave index == k-tile), so the
                        # lagging AV matmuls never outrank-starve the sigmoid
                        # pipeline
                        if qc == 0 and hp == 0 and wi in (0, 4, 8):
                            proj_v(1 + wi // 4)
                        st = ps_pool.tile([P, WAVE, 512], F32, tag="scores",
                                       bufs=2, name=f"st_{qc}_{hp}_{wi}")
                        for j, (kt, h) in enumerate(wave):
                            # scores^T tile: [k-tokens, q-tokens] for head
                            # 2hp+h; contraction over d (64 rows)
                            nc.tensor.matmul(
                                st[:, j, :],
                                lhsT=kt_sb[h * HD:(h + 1) * HD, hp,
                                           kt * P:(kt + 1) * P],
                                rhs=qt_tiles[qc][h * HD:(h + 1) * HD, hp, :],
                                start=True,
                                stop=True,
                                tile_position=(h * HD, 0),
                            )
                        a_t = apool.tile([P, WAVE, 512], F32R, tag="a_t",
                                         name=f"a_{qc}_{hp}_{wi}")
                        nc.scalar.activation(
                            out=a_t[:, :len(wave), :],
                            in_=st[:, :len(wave), :],
                            func=mybir.ActivationFunctionType.Sigmoid,
                            scale=1.0 / np.sqrt(HD).item(),
                        )
                        if pending is not None:
                            emit_avs(*pending)
                        pending = (wave, a_t)
                    emit_avs(*pending)
                    o_sb = opool.tile([P, 512], F32, tag="o_sb",
                                      name=f"osb_{qc}_{hp}")
                    nc.vector.tensor_copy(out=o_sb[0:HD, :], in_=o_accs[0][:])
                    nc.vector.tensor_copy(out=o_sb[HD:P, :], in_=o_accs[1][:])
                    nc.sync.dma_start(
                        o_t[hp * P:(hp + 1) * P, qc * 512:(qc + 1) * 512],
                        o_sb[:])

    nc.compile()
    return nc


def _prep_core_inputs(q, k, v, Wq, bq, Wk, bk, Wv, bv, KT):
    """Host-side shard + transpose. Returns in_maps for 8 cores."""
    KA = KT * P
    aug = KA > D

    def x_t(x_b):  # [S, D] -> [KA, S]
        xt = np.ascontiguousarray(x_b.T)  # [D, S]
        if not aug:
            return xt
        out = np.zeros((KA, S), np.float32)
        out[:D] = xt
        out[D] = 1.0
        return out

    def w_slice(W, b, half):  # -> [KA, OF]
        ws = W[:, half * OF:(half + 1) * OF]
        if not aug:
            return np.ascontiguousarray(ws)
        out = np.zeros((KA, OF), np.float32)
        out[:D] = ws
        out[D] = b[half * OF:(half + 1) * OF]
        return out

    xts = {}
    in_maps = []
    for c in range(N_CORES):
        b, half = divmod(c, 2)
        if b not in xts:
            xts[b] = (x_t(q[b]), x_t(k[b]), x_t(v[b]))
        xq_c, xk_c, xv_c = xts[b]
        in_maps.append({
            "xq": xq_c,
            "xk": xk_c,
            "xv": xv_c,
            "wq": w_slice(Wq, bq, half),
            "wk": w_slice(Wk, bk, half),
            "wv": w_slice(Wv, bv, half),
        })
    return in_maps


def kernel(q, k, v, Wq, bq, Wk, bk, Wv, bv):
    global last_results
    q = np.ascontiguousarray(np.asarray(q, np.float32))
    k = np.ascontiguousarray(np.asarray(k, np.float32))
    v = np.ascontiguousarray(np.asarray(v, np.float32))
    Wq = np.asarray(Wq, np.float32)
    Wk = np.asarray(Wk, np.float32)
    Wv = np.asarray(Wv, np.float32)
    bq = np.asarray(bq, np.float32)
    bk = np.asarray(bk, np.float32)
    bv = np.asarray(bv, np.float32)

    aug = any(np.any(b_) for b_ in (bq, bk, bv))
    KT = (D // P) + (1 if aug else 0)

    if KT not in _cache:
        _cache[KT] = _build(KT)
    nc = _cache[KT]

    in_maps = _prep_core_inputs(q, k, v, Wq, bq, Wk, bk, Wv, bv, KT)
    res = run_bass_kernel_spmd(nc, in_maps, core_ids=list(range(N_CORES)))
    last_results = res

    out = np.empty((B, S, D), np.float32)
    for c in range(N_CORES):
        b, half = divmod(c, 2)
        out[b, :, half * OF:(half + 1) * OF] = res.results[c]["o_t"].T
    return out
